# revision 17
# baseline (speedup 1.0000x reference)
"""AffineCoupling TRN2 kernel (v6.5): 31 macros of 16384 rows per core.

Same dataflow as v7 but at 16k-row macro granularity: SWDGE cast DMAs,
transpose-mode fwd/back transposes into bf16 PSUM, A/B column-half MLP
ping-pong between PE and ACT, packed full-lane L5/Exp, in-place combine.
PSUM: hA(2) + hB(2) + x0ps(2) + eT(1) + bT(1) = 8 banks.
"""
import os
import sys

sys.path.insert(0, "/opt/trn_rl_repo")
if "/root/.axon_site/_ro/trn_rl_repo" not in sys.path:
    sys.path.append("/root/.axon_site/_ro/trn_rl_repo")

import numpy as np

import concourse.bacc as bacc
import concourse.bass as bass
import concourse.tile as tile
from concourse import mybir
from concourse.bass import _add_dep_helper
from concourse.bass_utils import run_bass_kernel_spmd

FP = mybir.dt.float32
BF = mybir.dt.bfloat16

N_CORES = 8
BATCH = 4_000_000
ROWS_PER_MACRO = 16_384
MACROS = 31
R = ROWS_PER_MACRO * MACROS        # 507,904 rows per core
PAD_ROWS = ROWS_PER_MACRO

STEP = 498_688
STARTS = [c * STEP for c in range(N_CORES - 1)] + [BATCH - R]

C_BIAS = 128
C_TOTAL = 135

LAST_RESULTS = None


def _build_consts(ws_logs, bs_logs, ws_b, bs_b):
    import ml_dtypes

    ws_logs = [np.asarray(w, np.float32) for w in ws_logs]
    bs_logs = [np.asarray(b, np.float32) for b in bs_logs]
    ws_b = [np.asarray(w, np.float32) for w in ws_b]
    bs_b = [np.asarray(b, np.float32) for b in bs_b]

    consts = np.zeros((128, C_TOTAL), np.float32)
    consts[:, 0:128] = np.eye(128, dtype=np.float32)
    for k in range(4):
        cat = np.concatenate([bs_logs[k], bs_b[k]])    # [16]
        consts[:, C_BIAS + k] = np.tile(cat, 8)
    consts[:, C_BIAS + 4] = np.concatenate(
        [np.tile(bs_logs[4], 8), np.tile(bs_b[4], 8)]
    )
    consts[:, C_BIAS + 5] = np.tile(bs_logs[4], 16)
    consts[:, C_BIAS + 6] = np.tile(bs_b[4], 16)

    wmat = np.zeros((128, 5 * 128), np.float32)
    w1cat = np.vstack([ws_logs[0], ws_b[0]])           # [16, 8]
    for g in range(8):
        wmat[g * 16:g * 16 + 8, g * 16:(g + 1) * 16] = w1cat.T
    for k in (1, 2, 3):
        wk = np.zeros((16, 16), np.float32)
        wk[0:8, 0:8] = ws_logs[k]
        wk[8:16, 8:16] = ws_b[k]
        for g in range(8):
            wmat[g * 16:(g + 1) * 16, k * 128 + g * 16:k * 128 + (g + 1) * 16] = wk.T
    for g in range(8):
        wmat[g * 16:g * 16 + 8, 4 * 128 + g * 8:4 * 128 + (g + 1) * 8] = ws_logs[4].T
        wmat[g * 16 + 8:(g + 1) * 16,
             4 * 128 + 64 + g * 8:4 * 128 + 64 + (g + 1) * 8] = ws_b[4].T
    wmat = np.concatenate([wmat, np.eye(128, dtype=np.float32)], axis=1)
    wmat_bf = wmat.astype(ml_dtypes.bfloat16)
    return consts, wmat_bf


def _ap(t, offset, dims):
    return bass.AP(tensor=t.tensor, offset=t.offset + offset, ap=[t.ap[0]] + dims)


def _build_nc():
    nc = bacc.Bacc()
    z_d = nc.declare_dram_parameter("z", [R + PAD_ROWS, 16], FP, isOutput=False)
    c_d = nc.declare_dram_parameter("consts", [128, C_TOTAL], FP, isOutput=False)
    w_d = nc.declare_dram_parameter("wmat", [128, 6 * 128], BF, isOutput=False)
    o_d = nc.declare_dram_parameter("out", [R + PAD_ROWS, 16], FP, isOutput=True)

    with tile.TileContext(nc) as tc:
        with (
            tc.tile_pool(name="consts", bufs=1) as cp,
            tc.tile_pool(name="nat", bufs=1) as natp,
            tc.tile_pool(name="sb", bufs=1) as sbp,
            tc.tile_pool(name="ps", bufs=1, space="PSUM") as psp,
        ):
            consts = cp.tile([128, C_TOTAL], FP)
            nc.sync.dma_start(out=consts, in_=c_d[:, :])
            wmat = cp.tile([128, 6 * 128], BF)
            nc.sync.dma_start(out=wmat, in_=w_d[:, :])
            identbf = wmat[:, 5 * 128:6 * 128]
            lhsT = [wmat[:, k * 128:(k + 1) * 128] for k in range(5)]
            biases = [consts[:, C_BIAS + k:C_BIAS + k + 1] for k in range(7)]

            wu1 = sbp.tile([128, 1], FP, tag="wu")
            nc.scalar.copy(out=wu1, in_=biases[0])
            wu2 = sbp.tile([128, 1], FP, tag="wu")
            nc.vector.tensor_copy(out=wu2, in_=biases[0])

            natbfs = {}
            tail_dmas = []

            def load(m):
                if m >= MACROS:
                    return
                r0 = m * ROWS_PER_MACRO
                natbf = natp.tile([128, 2048], BF, tag="nat", bufs=5)
                nc.gpsimd.dma_start(
                    out=natbf.rearrange("p (c f) -> p c f", c=128, f=16),
                    in_=z_d[r0:r0 + ROWS_PER_MACRO, :].rearrange(
                        "(p c) f -> p c f", p=128, c=128
                    ),
                )
                natbfs[m] = natbf

            def fwdT_quarter(m, q, x0ps):
                for u in range(q * 4, q * 4 + 4):
                    nc.tensor.transpose(
                        x0ps[:, u * 128:(u + 1) * 128],
                        natbfs[m][:, u * 128:(u + 1) * 128],
                        identbf,
                    )

            def half_mms(lhsT_k, h_in, half, tag):
                hps = psp.tile([128, 1024], FP, tag=tag, bufs=1)
                for n in range(2):
                    src = h_in[:, half * 1024 + n * 512:half * 1024 + (n + 1) * 512]
                    nc.tensor.matmul(hps[:, n * 512:(n + 1) * 512],
                                     lhsT_k, src, start=True, stop=True)
                return hps

            def half_prelu(hps, k, hb, half):
                nc.scalar.activation(
                    out=hb[:, half * 1024:(half + 1) * 1024], in_=hps,
                    func=mybir.ActivationFunctionType.Prelu,
                    bias=biases[k], scale=1.0, alpha=0.01,
                )

            def backT_quarter(eT, bT, eb, q):
                for u in range(q * 2, q * 2 + 2):
                    nc.tensor.transpose(
                        eT[:, u * 128:(u + 1) * 128],
                        eb[:, u * 128:(u + 1) * 128],
                        identbf,
                    )
                    nc.tensor.transpose(
                        bT[:, u * 128:(u + 1) * 128],
                        eb[:, 1024 + u * 128:1024 + (u + 1) * 128],
                        identbf,
                    )

            def combine_quarter(eT, bT, natbf, Q):
                # natbf cols [Q*512, (Q+1)*512): eT/bT offset by sub-half
                eoff = (Q // 2) * 512 + (Q % 2) * 64
                e_ap = _ap(eT, eoff, [[128, 4], [8, 8], [1, 8]])
                b_ap = _ap(bT, eoff, [[128, 4], [8, 8], [1, 8]])
                zr_ap = _ap(natbf, Q * 512 + 8, [[128, 4], [16, 8], [1, 8]])
                tmp = sbp.tile([128, 512], BF, tag="tmp", bufs=2)
                tmp_ap = _ap(tmp, 0, [[128, 4], [8, 8], [1, 8]])
                nc.vector.tensor_mul(out=tmp_ap, in0=e_ap, in1=zr_ap)
                nc.vector.tensor_add(out=zr_ap, in0=tmp_ap, in1=b_ap)

            def store(m):
                r0 = m * ROWS_PER_MACRO
                out_dma = nc.gpsimd.dma_start(
                    out=o_d[r0:r0 + ROWS_PER_MACRO, :].rearrange(
                        "(p c) f -> p c f", p=128, c=128
                    ),
                    in_=natbfs[m].rearrange("p (c f) -> p c f", c=128, f=16),
                )
                del natbfs[m]
                load(m + 3)
                if m >= MACROS - 4:
                    tail_dmas.append(out_dma)

            def layer1(m, x0):
                hb = sbp.tile([128, 2048], BF, tag="h0", bufs=2)
                hA = half_mms(lhsT[0], x0, 0, "hA")
                hB = half_mms(lhsT[0], x0, 1, "hB")
                half_prelu(hA, 0, hb, 0)
                half_prelu(hB, 0, hb, 1)
                return hb

            load(0)
            load(1)
            load(2)
            x0ps = psp.tile([128, 2048], BF, tag="x0ps", bufs=1)
            for q in range(4):
                fwdT_quarter(0, q, x0ps)
            x0 = sbp.tile([128, 2048], BF, tag="x0", bufs=2)
            nc.vector.tensor_copy(out=x0, in_=x0ps)
            h = layer1(0, x0)

            for m in range(MACROS):
                nxt = m + 1 < MACROS
                eb = sbp.tile([128, 2048], BF, tag="eb", bufs=2)
                if nxt:
                    x0ps = psp.tile([128, 2048], BF, tag="x0ps", bufs=1)
                if nxt:
                    x0 = sbp.tile([128, 2048], BF, tag="x0", bufs=2)
                for k in (1, 2, 3):
                    hb = sbp.tile([128, 2048], BF, tag=f"h{k}", bufs=2)
                    hA = half_mms(lhsT[k], h, 0, "hA")
                    hB = half_mms(lhsT[k], h, 1, "hB")
                    if nxt:
                        fwdT_quarter(m + 1, k - 1, x0ps)
                        if k == 2:
                            nc.vector.tensor_copy(out=x0[:, 0:1024],
                                                  in_=x0ps[:, 0:1024])
                    half_prelu(hA, k, hb, 0)
                    half_prelu(hB, k, hb, 1)
                    h = hb
                # L5, column-split packing: gen half H holds
                #   cols 0:512  = e  (partitions 0:64 <- h cols H*1024+0:512,
                #                      64:128 <- h cols H*1024+512:1024)
                #   cols 512:1024 = b (same split)
                # so Exp-A depends only on Prelu-A of L4. The (0,0)/(0,64)
                # col-tiled MM pairs run concurrently on the PE.
                lhsT5e = lhsT[4][:, 0:64]
                lhsT5b = lhsT[4][:, 64:128]
                gens = []
                for H, tag in ((0, "hA"), (1, "hB")):
                    gen = psp.tile([128, 1024], FP, tag=tag, bufs=1)
                    gens.append(gen)
                    for n in range(2):
                        src = h[:, H * 1024 + n * 512:H * 1024 + (n + 1) * 512]
                        nc.tensor.matmul(gen[64 * n:64 * n + 64, 0:512],
                                         lhsT5e, src, start=True, stop=True)
                    for n in range(2):
                        src = h[:, H * 1024 + n * 512:H * 1024 + (n + 1) * 512]
                        nc.tensor.matmul(gen[64 * n:64 * n + 64, 512:1024],
                                         lhsT5b, src, start=True, stop=True)
                    nc.vector.tensor_scalar_add(
                        out=eb[:, 1024 + H * 512:1024 + (H + 1) * 512],
                        in0=gen[:, 512:1024], scalar1=biases[6],
                    )
                    if H == 0:
                        if nxt:
                            fwdT_quarter(m + 1, 3, x0ps)
                            nc.vector.tensor_copy(out=x0[:, 1024:2048],
                                                  in_=x0ps[:, 1024:2048])
                    nc.scalar.activation(
                        out=eb[:, H * 512:(H + 1) * 512], in_=gen[:, 0:512],
                        func=mybir.ActivationFunctionType.Exp,
                        bias=biases[5], scale=1.0,
                    )
                if nxt:
                    h = layer1(m + 1, x0)

                eT = psp.tile([128, 1024], BF, tag="eT", bufs=1)
                bT = psp.tile([128, 1024], BF, tag="bT", bufs=1)
                for q in range(4):
                    backT_quarter(eT, bT, eb, q)
                for Q in range(4):
                    combine_quarter(eT, bT, natbfs[m], Q)
                store(m)

            flush = sbp.tile([128, 1], FP, tag="wu")
            fl = nc.vector.tensor_copy(out=flush, in_=biases[0])
            for dma in tail_dmas:
                _add_dep_helper(fl.ins, dma.ins, sync=True,
                                reason="drain tail out-DMAs before kernel end")

    nc.finalize()
    return nc


_NC_CACHE = None


def kernel(z, ws_logs, bs_logs, ws_b, bs_b):
    global _NC_CACHE, LAST_RESULTS
    z = np.asarray(z, np.float32)
    assert z.shape == (BATCH, 16)
    consts, wmat_bf = _build_consts(ws_logs, bs_logs, ws_b, bs_b)

    if _NC_CACHE is None:
        _NC_CACHE = _build_nc()
    nc = _NC_CACHE

    in_maps = []
    for s in STARTS:
        zp = np.zeros((R + PAD_ROWS, 16), np.float32)
        zp[:R] = z[s:s + R]
        in_maps.append({"z": zp, "consts": consts, "wmat": wmat_bf})
    trace = bool(os.environ.get("AFFINE_TRACE"))
    res = run_bass_kernel_spmd(nc, in_maps, core_ids=list(range(N_CORES)), trace=trace)
    LAST_RESULTS = res

    out = np.empty((BATCH, 16), np.float32)
    for c in range(N_CORES):
        out[STARTS[c]:STARTS[c] + R] = res.results[c]["out"][:R]
    return out


# revision 18
# speedup vs baseline: 1.1100x; 1.1100x over previous
"""AffineCoupling TRN2 kernel (v6.5): 31 macros of 16384 rows per core.

Same dataflow as v7 but at 16k-row macro granularity: SWDGE cast DMAs,
transpose-mode fwd/back transposes into bf16 PSUM, A/B column-half MLP
ping-pong between PE and ACT, packed full-lane L5/Exp, in-place combine.
PSUM: hA(2) + hB(2) + x0ps(2) + eT(1) + bT(1) = 8 banks.
"""
import os
import sys

sys.path.insert(0, "/opt/trn_rl_repo")
if "/root/.axon_site/_ro/trn_rl_repo" not in sys.path:
    sys.path.append("/root/.axon_site/_ro/trn_rl_repo")

import numpy as np

import concourse.bacc as bacc
import concourse.bass as bass
import concourse.tile as tile
from concourse import mybir
from concourse.bass import _add_dep_helper
from concourse.bass_utils import run_bass_kernel_spmd

FP = mybir.dt.float32
BF = mybir.dt.bfloat16

N_CORES = 8
BATCH = 4_000_000
ROWS_PER_MACRO = 16_384
MACROS = 31
R = ROWS_PER_MACRO * MACROS        # 507,904 rows per core
PAD_ROWS = ROWS_PER_MACRO

STEP = 498_688
STARTS = [c * STEP for c in range(N_CORES - 1)] + [BATCH - R]

C_BIAS = 128
C_TOTAL = 135

LAST_RESULTS = None


def _build_consts(ws_logs, bs_logs, ws_b, bs_b):
    import ml_dtypes

    ws_logs = [np.asarray(w, np.float32) for w in ws_logs]
    bs_logs = [np.asarray(b, np.float32) for b in bs_logs]
    ws_b = [np.asarray(w, np.float32) for w in ws_b]
    bs_b = [np.asarray(b, np.float32) for b in bs_b]

    consts = np.zeros((128, C_TOTAL), np.float32)
    consts[:, 0:128] = np.eye(128, dtype=np.float32)
    for k in range(4):
        cat = np.concatenate([bs_logs[k], bs_b[k]])    # [16]
        consts[:, C_BIAS + k] = np.tile(cat, 8)
    consts[:, C_BIAS + 4] = np.concatenate(
        [np.tile(bs_logs[4], 8), np.tile(bs_b[4], 8)]
    )
    consts[:, C_BIAS + 5] = np.tile(bs_logs[4], 16)
    consts[:, C_BIAS + 6] = np.tile(bs_b[4], 16)

    wmat = np.zeros((128, 5 * 128), np.float32)
    w1cat = np.vstack([ws_logs[0], ws_b[0]])           # [16, 8]
    for g in range(8):
        wmat[g * 16:g * 16 + 8, g * 16:(g + 1) * 16] = w1cat.T
    for k in (1, 2, 3):
        wk = np.zeros((16, 16), np.float32)
        wk[0:8, 0:8] = ws_logs[k]
        wk[8:16, 8:16] = ws_b[k]
        for g in range(8):
            wmat[g * 16:(g + 1) * 16, k * 128 + g * 16:k * 128 + (g + 1) * 16] = wk.T
    for g in range(8):
        wmat[g * 16:g * 16 + 8, 4 * 128 + g * 8:4 * 128 + (g + 1) * 8] = ws_logs[4].T
        wmat[g * 16 + 8:(g + 1) * 16,
             4 * 128 + 64 + g * 8:4 * 128 + 64 + (g + 1) * 8] = ws_b[4].T
    wmat = np.concatenate([wmat, np.eye(128, dtype=np.float32)], axis=1)
    wmat_bf = wmat.astype(ml_dtypes.bfloat16)
    return consts, wmat_bf


def _ap(t, offset, dims):
    return bass.AP(tensor=t.tensor, offset=t.offset + offset, ap=[t.ap[0]] + dims)


def _build_nc():
    nc = bacc.Bacc()
    z_d = nc.declare_dram_parameter("z", [R + PAD_ROWS, 16], FP, isOutput=False)
    c_d = nc.declare_dram_parameter("consts", [128, C_TOTAL], FP, isOutput=False)
    w_d = nc.declare_dram_parameter("wmat", [128, 6 * 128], BF, isOutput=False)
    o_d = nc.declare_dram_parameter("out", [R + PAD_ROWS, 16], FP, isOutput=True)

    with tile.TileContext(nc) as tc:
        with (
            tc.tile_pool(name="consts", bufs=1) as cp,
            tc.tile_pool(name="nat", bufs=1) as natp,
            tc.tile_pool(name="sb", bufs=1) as sbp,
            tc.tile_pool(name="ps", bufs=1, space="PSUM") as psp,
        ):
            consts = cp.tile([128, C_TOTAL], FP)
            nc.sync.dma_start(out=consts, in_=c_d[:, :])
            wmat = cp.tile([128, 6 * 128], BF)
            nc.sync.dma_start(out=wmat, in_=w_d[:, :])
            identbf = wmat[:, 5 * 128:6 * 128]
            lhsT = [wmat[:, k * 128:(k + 1) * 128] for k in range(5)]
            biases = [consts[:, C_BIAS + k:C_BIAS + k + 1] for k in range(7)]

            wu1 = sbp.tile([128, 1], FP, tag="wu")
            nc.scalar.copy(out=wu1, in_=biases[0])
            wu2 = sbp.tile([128, 1], FP, tag="wu")
            nc.vector.tensor_copy(out=wu2, in_=biases[0])

            natbfs = {}
            tail_dmas = []

            def load(m):
                if m >= MACROS:
                    return
                r0 = m * ROWS_PER_MACRO
                natbf = natp.tile([128, 2048], BF, tag="nat", bufs=5)
                nc.gpsimd.dma_start(
                    out=natbf.rearrange("p (c f) -> p c f", c=128, f=16),
                    in_=z_d[r0:r0 + ROWS_PER_MACRO, :].rearrange(
                        "(p c) f -> p c f", p=128, c=128
                    ),
                )
                natbfs[m] = natbf

            def fwdT_quarter(m, q, x0ps):
                for u in range(q * 4, q * 4 + 4):
                    nc.tensor.transpose(
                        x0ps[:, u * 128:(u + 1) * 128],
                        natbfs[m][:, u * 128:(u + 1) * 128],
                        identbf,
                    )

            def half_mms(lhsT_k, h_in, half, tag):
                hps = psp.tile([128, 1024], FP, tag=tag, bufs=1)
                for n in range(2):
                    src = h_in[:, half * 1024 + n * 512:half * 1024 + (n + 1) * 512]
                    nc.tensor.matmul(hps[:, n * 512:(n + 1) * 512],
                                     lhsT_k, src, start=True, stop=True)
                return hps

            def half_prelu(hps, k, hb, half):
                nc.scalar.activation(
                    out=hb[:, half * 1024:(half + 1) * 1024], in_=hps,
                    func=mybir.ActivationFunctionType.Prelu,
                    bias=biases[k], scale=1.0, alpha=0.01,
                )

            def backT_quarter(eT, bT, eb, q):
                for u in range(q * 2, q * 2 + 2):
                    nc.tensor.transpose(
                        eT[:, u * 128:(u + 1) * 128],
                        eb[:, u * 128:(u + 1) * 128],
                        identbf,
                    )
                    nc.tensor.transpose(
                        bT[:, u * 128:(u + 1) * 128],
                        eb[:, 1024 + u * 128:1024 + (u + 1) * 128],
                        identbf,
                    )

            def combine_quarter(eT, bT, natbf, Q):
                # natbf cols [Q*512, (Q+1)*512): eT/bT offset by sub-half
                eoff = (Q // 2) * 512 + (Q % 2) * 64
                e_ap = _ap(eT, eoff, [[128, 4], [8, 8], [1, 8]])
                b_ap = _ap(bT, eoff, [[128, 4], [8, 8], [1, 8]])
                zr_ap = _ap(natbf, Q * 512 + 8, [[128, 4], [16, 8], [1, 8]])
                tmp = sbp.tile([128, 512], BF, tag="tmp", bufs=2)
                tmp_ap = _ap(tmp, 0, [[128, 4], [8, 8], [1, 8]])
                nc.vector.tensor_mul(out=tmp_ap, in0=e_ap, in1=zr_ap)
                nc.vector.tensor_add(out=zr_ap, in0=tmp_ap, in1=b_ap)

            def store(m):
                r0 = m * ROWS_PER_MACRO
                out_dma = nc.gpsimd.dma_start(
                    out=o_d[r0:r0 + ROWS_PER_MACRO, :].rearrange(
                        "(p c) f -> p c f", p=128, c=128
                    ),
                    in_=natbfs[m].rearrange("p (c f) -> p c f", c=128, f=16),
                )
                del natbfs[m]
                load(m + 3)
                if m >= MACROS - 4:
                    tail_dmas.append(out_dma)

            def layer1(m, x0):
                hb = sbp.tile([128, 2048], BF, tag="h0", bufs=2)
                hA = half_mms(lhsT[0], x0, 0, "hA")
                hB = half_mms(lhsT[0], x0, 1, "hB")
                half_prelu(hA, 0, hb, 0)
                half_prelu(hB, 0, hb, 1)
                return hb

            load(0)
            load(1)
            load(2)
            x0ps = psp.tile([128, 2048], BF, tag="x0ps", bufs=1)
            for q in range(4):
                fwdT_quarter(0, q, x0ps)
            x0 = sbp.tile([128, 2048], BF, tag="x0", bufs=2)
            nc.vector.tensor_copy(out=x0, in_=x0ps)
            h = layer1(0, x0)

            for m in range(MACROS):
                nxt = m + 1 < MACROS
                eb = sbp.tile([128, 2048], BF, tag="eb", bufs=2)
                if nxt:
                    x0ps = psp.tile([128, 2048], BF, tag="x0ps", bufs=1)
                if nxt:
                    x0 = sbp.tile([128, 2048], BF, tag="x0", bufs=2)
                for k in (1, 2, 3):
                    hb = sbp.tile([128, 2048], BF, tag=f"h{k}", bufs=2)
                    hA = half_mms(lhsT[k], h, 0, "hA")
                    hB = half_mms(lhsT[k], h, 1, "hB")
                    if nxt:
                        fwdT_quarter(m + 1, k - 1, x0ps)
                        if k == 2:
                            nc.vector.tensor_copy(out=x0[:, 0:1024],
                                                  in_=x0ps[:, 0:1024])
                        elif k == 3:
                            fwdT_quarter(m + 1, 3, x0ps)
                            nc.vector.tensor_copy(out=x0[:, 1024:2048],
                                                  in_=x0ps[:, 1024:2048])
                    half_prelu(hA, k, hb, 0)
                    half_prelu(hB, k, hb, 1)
                    h = hb
                if nxt:
                    # L1 of macro m+1 ahead of L5: it is on the ping-pong
                    # critical path, L5/Exp only feed the (slack) store.
                    h_next = layer1(m + 1, x0)
                # L5, column-split packing: gen half H holds
                #   cols 0:512  = e  (partitions 0:64 <- h cols H*1024+0:512,
                #                      64:128 <- h cols H*1024+512:1024)
                #   cols 512:1024 = b (same split)
                # so Exp-A depends only on Prelu-A of L4. The (0,0)/(0,64)
                # col-tiled MM pairs run concurrently on the PE.
                lhsT5e = lhsT[4][:, 0:64]
                lhsT5b = lhsT[4][:, 64:128]
                gens = []
                for H, tag in ((0, "hA"), (1, "hB")):
                    gen = psp.tile([128, 1024], FP, tag=tag, bufs=1)
                    gens.append(gen)
                    for n in range(2):
                        src = h[:, H * 1024 + n * 512:H * 1024 + (n + 1) * 512]
                        nc.tensor.matmul(gen[64 * n:64 * n + 64, 0:512],
                                         lhsT5e, src, start=True, stop=True)
                    for n in range(2):
                        src = h[:, H * 1024 + n * 512:H * 1024 + (n + 1) * 512]
                        nc.tensor.matmul(gen[64 * n:64 * n + 64, 512:1024],
                                         lhsT5b, src, start=True, stop=True)
                    nc.scalar.activation(
                        out=eb[:, H * 512:(H + 1) * 512], in_=gen[:, 0:512],
                        func=mybir.ActivationFunctionType.Exp,
                        bias=biases[5], scale=1.0,
                    )
                    nc.vector.tensor_scalar_add(
                        out=eb[:, 1024 + H * 512:1024 + (H + 1) * 512],
                        in0=gen[:, 512:1024], scalar1=biases[6],
                    )
                if nxt:
                    h = h_next

                eT = psp.tile([128, 1024], BF, tag="eT", bufs=1)
                bT = psp.tile([128, 1024], BF, tag="bT", bufs=1)
                for q in range(4):
                    backT_quarter(eT, bT, eb, q)
                for Q in range(4):
                    combine_quarter(eT, bT, natbfs[m], Q)
                store(m)

            flush = sbp.tile([128, 1], FP, tag="wu")
            fl = nc.vector.tensor_copy(out=flush, in_=biases[0])
            for dma in tail_dmas:
                _add_dep_helper(fl.ins, dma.ins, sync=True,
                                reason="drain tail out-DMAs before kernel end")

    nc.finalize()
    return nc


_NC_CACHE = None


def kernel(z, ws_logs, bs_logs, ws_b, bs_b):
    global _NC_CACHE, LAST_RESULTS
    z = np.asarray(z, np.float32)
    assert z.shape == (BATCH, 16)
    consts, wmat_bf = _build_consts(ws_logs, bs_logs, ws_b, bs_b)

    if _NC_CACHE is None:
        _NC_CACHE = _build_nc()
    nc = _NC_CACHE

    in_maps = []
    for s in STARTS:
        zp = np.zeros((R + PAD_ROWS, 16), np.float32)
        zp[:R] = z[s:s + R]
        in_maps.append({"z": zp, "consts": consts, "wmat": wmat_bf})
    trace = bool(os.environ.get("AFFINE_TRACE"))
    res = run_bass_kernel_spmd(nc, in_maps, core_ids=list(range(N_CORES)), trace=trace)
    LAST_RESULTS = res

    out = np.empty((BATCH, 16), np.float32)
    for c in range(N_CORES):
        out[STARTS[c]:STARTS[c] + R] = res.results[c]["out"][:R]
    return out


# revision 19
# speedup vs baseline: 1.1318x; 1.0197x over previous
"""AffineCoupling TRN2 kernel (v6.5): 31 macros of 16384 rows per core.

Same dataflow as v7 but at 16k-row macro granularity: SWDGE cast DMAs,
transpose-mode fwd/back transposes into bf16 PSUM, A/B column-half MLP
ping-pong between PE and ACT, packed full-lane L5/Exp, in-place combine.
PSUM: hA(2) + hB(2) + x0ps(2) + eT(1) + bT(1) = 8 banks.
"""
import os
import sys

sys.path.insert(0, "/opt/trn_rl_repo")
if "/root/.axon_site/_ro/trn_rl_repo" not in sys.path:
    sys.path.append("/root/.axon_site/_ro/trn_rl_repo")

import numpy as np

import concourse.bacc as bacc
import concourse.bass as bass
import concourse.tile as tile
from concourse import mybir
from concourse.bass import _add_dep_helper
from concourse.bass_utils import run_bass_kernel_spmd

FP = mybir.dt.float32
BF = mybir.dt.bfloat16

N_CORES = 8
BATCH = 4_000_000
ROWS_PER_MACRO = 16_384
MACROS = 31
R = ROWS_PER_MACRO * MACROS        # 507,904 rows per core
PAD_ROWS = ROWS_PER_MACRO

STEP = 498_688
STARTS = [c * STEP for c in range(N_CORES - 1)] + [BATCH - R]

C_BIAS = 128
C_TOTAL = 135

LAST_RESULTS = None


def _build_consts(ws_logs, bs_logs, ws_b, bs_b):
    import ml_dtypes

    ws_logs = [np.asarray(w, np.float32) for w in ws_logs]
    bs_logs = [np.asarray(b, np.float32) for b in bs_logs]
    ws_b = [np.asarray(w, np.float32) for w in ws_b]
    bs_b = [np.asarray(b, np.float32) for b in bs_b]

    consts = np.zeros((128, C_TOTAL), np.float32)
    consts[:, 0:128] = np.eye(128, dtype=np.float32)
    for k in range(4):
        cat = np.concatenate([bs_logs[k], bs_b[k]])    # [16]
        consts[:, C_BIAS + k] = np.tile(cat, 8)
    consts[:, C_BIAS + 4] = np.concatenate(
        [np.tile(bs_logs[4], 8), np.tile(bs_b[4], 8)]
    )
    consts[:, C_BIAS + 5] = np.tile(bs_logs[4], 16)
    consts[:, C_BIAS + 6] = np.tile(bs_b[4], 16)

    wmat = np.zeros((128, 5 * 128), np.float32)
    w1cat = np.vstack([ws_logs[0], ws_b[0]])           # [16, 8]
    for g in range(8):
        wmat[g * 16:g * 16 + 8, g * 16:(g + 1) * 16] = w1cat.T
    for k in (1, 2, 3):
        wk = np.zeros((16, 16), np.float32)
        wk[0:8, 0:8] = ws_logs[k]
        wk[8:16, 8:16] = ws_b[k]
        for g in range(8):
            wmat[g * 16:(g + 1) * 16, k * 128 + g * 16:k * 128 + (g + 1) * 16] = wk.T
    for g in range(8):
        wmat[g * 16:g * 16 + 8, 4 * 128 + g * 8:4 * 128 + (g + 1) * 8] = ws_logs[4].T
        wmat[g * 16 + 8:(g + 1) * 16,
             4 * 128 + 64 + g * 8:4 * 128 + 64 + (g + 1) * 8] = ws_b[4].T
    wmat = np.concatenate([wmat, np.eye(128, dtype=np.float32)], axis=1)
    wmat_bf = wmat.astype(ml_dtypes.bfloat16)
    return consts, wmat_bf


def _ap(t, offset, dims):
    return bass.AP(tensor=t.tensor, offset=t.offset + offset, ap=[t.ap[0]] + dims)


def _build_nc():
    nc = bacc.Bacc()
    z_d = nc.declare_dram_parameter("z", [R + PAD_ROWS, 16], FP, isOutput=False)
    c_d = nc.declare_dram_parameter("consts", [128, C_TOTAL], FP, isOutput=False)
    w_d = nc.declare_dram_parameter("wmat", [128, 6 * 128], BF, isOutput=False)
    o_d = nc.declare_dram_parameter("out", [R + PAD_ROWS, 16], FP, isOutput=True)

    with tile.TileContext(nc) as tc:
        with (
            tc.tile_pool(name="consts", bufs=1) as cp,
            tc.tile_pool(name="nat", bufs=1) as natp,
            tc.tile_pool(name="sb", bufs=1) as sbp,
            tc.tile_pool(name="ps", bufs=1, space="PSUM") as psp,
        ):
            consts = cp.tile([128, C_TOTAL], FP)
            nc.sync.dma_start(out=consts, in_=c_d[:, :])
            wmat = cp.tile([128, 6 * 128], BF)
            nc.sync.dma_start(out=wmat, in_=w_d[:, :])
            identbf = wmat[:, 5 * 128:6 * 128]
            lhsT = [wmat[:, k * 128:(k + 1) * 128] for k in range(5)]
            biases = [consts[:, C_BIAS + k:C_BIAS + k + 1] for k in range(7)]

            wu1 = sbp.tile([128, 1], FP, tag="wu")
            nc.scalar.copy(out=wu1, in_=biases[0])
            wu2 = sbp.tile([128, 1], FP, tag="wu")
            nc.vector.tensor_copy(out=wu2, in_=biases[0])

            natbfs = {}
            tail_dmas = []

            def load(m):
                if m >= MACROS:
                    return
                r0 = m * ROWS_PER_MACRO
                natbf = natp.tile([128, 2048], BF, tag="nat", bufs=5)
                nc.gpsimd.dma_start(
                    out=natbf.rearrange("p (c f) -> p c f", c=128, f=16),
                    in_=z_d[r0:r0 + ROWS_PER_MACRO, :].rearrange(
                        "(p c) f -> p c f", p=128, c=128
                    ),
                )
                natbfs[m] = natbf

            def fwdT_quarter(m, q, x0ps):
                for u in range(q * 4, q * 4 + 4):
                    nc.tensor.transpose(
                        x0ps[:, u * 128:(u + 1) * 128],
                        natbfs[m][:, u * 128:(u + 1) * 128],
                        identbf,
                    )

            def half_mms(lhsT_k, h_in, half, tag):
                hps = psp.tile([128, 1024], FP, tag=tag, bufs=1)
                for n in range(2):
                    src = h_in[:, half * 1024 + n * 512:half * 1024 + (n + 1) * 512]
                    nc.tensor.matmul(hps[:, n * 512:(n + 1) * 512],
                                     lhsT_k, src, start=True, stop=True)
                return hps

            def half_prelu(hps, k, hb, half):
                nc.scalar.activation(
                    out=hb[:, half * 1024:(half + 1) * 1024], in_=hps,
                    func=mybir.ActivationFunctionType.Prelu,
                    bias=biases[k], scale=1.0, alpha=0.01,
                )

            def backT_quarter(eT, bT, eb, q):
                for u in range(q * 2, q * 2 + 2):
                    nc.tensor.transpose(
                        eT[:, u * 128:(u + 1) * 128],
                        eb[:, u * 128:(u + 1) * 128],
                        identbf,
                    )
                    nc.tensor.transpose(
                        bT[:, u * 128:(u + 1) * 128],
                        eb[:, 1024 + u * 128:1024 + (u + 1) * 128],
                        identbf,
                    )

            def combine_quarter(eT, bT, natbf, Q):
                # natbf cols [Q*512, (Q+1)*512): eT/bT offset by sub-half
                eoff = (Q // 2) * 512 + (Q % 2) * 64
                e_ap = _ap(eT, eoff, [[128, 4], [8, 8], [1, 8]])
                b_ap = _ap(bT, eoff, [[128, 4], [8, 8], [1, 8]])
                zr_ap = _ap(natbf, Q * 512 + 8, [[128, 4], [16, 8], [1, 8]])
                tmp = sbp.tile([128, 512], BF, tag="tmp", bufs=2)
                tmp_ap = _ap(tmp, 0, [[128, 4], [8, 8], [1, 8]])
                nc.vector.tensor_mul(out=tmp_ap, in0=e_ap, in1=zr_ap)
                nc.vector.tensor_add(out=zr_ap, in0=tmp_ap, in1=b_ap)

            def store(m):
                r0 = m * ROWS_PER_MACRO
                out_dma = nc.gpsimd.dma_start(
                    out=o_d[r0:r0 + ROWS_PER_MACRO, :].rearrange(
                        "(p c) f -> p c f", p=128, c=128
                    ),
                    in_=natbfs[m].rearrange("p (c f) -> p c f", c=128, f=16),
                )
                del natbfs[m]
                load(m + 3)
                if m >= MACROS - 4:
                    tail_dmas.append(out_dma)

            def layer1(m, x0):
                hb = sbp.tile([128, 2048], BF, tag="h0", bufs=2)
                hA = half_mms(lhsT[0], x0, 0, "hA")
                hB = half_mms(lhsT[0], x0, 1, "hB")
                half_prelu(hA, 0, hb, 0)
                half_prelu(hB, 0, hb, 1)
                return hb

            load(0)
            load(1)
            load(2)
            x0ps = psp.tile([128, 2048], BF, tag="x0ps", bufs=1)
            for q in range(4):
                fwdT_quarter(0, q, x0ps)
            x0 = sbp.tile([128, 2048], BF, tag="x0", bufs=2)
            nc.vector.tensor_copy(out=x0, in_=x0ps)
            h = layer1(0, x0)

            for m in range(MACROS):
                nxt = m + 1 < MACROS
                eb = sbp.tile([128, 2048], BF, tag="eb", bufs=2)
                if nxt:
                    x0ps = psp.tile([128, 2048], BF, tag="x0ps", bufs=1)
                if nxt:
                    x0 = sbp.tile([128, 2048], BF, tag="x0", bufs=2)
                for k in (1, 2, 3):
                    hb = sbp.tile([128, 2048], BF, tag=f"h{k}", bufs=2)
                    hA = half_mms(lhsT[k], h, 0, "hA")
                    hB = half_mms(lhsT[k], h, 1, "hB")
                    if nxt:
                        fwdT_quarter(m + 1, k - 1, x0ps)
                        if k == 2:
                            nc.vector.tensor_copy(out=x0[:, 0:1024],
                                                  in_=x0ps[:, 0:1024])
                        elif k == 3:
                            fwdT_quarter(m + 1, 3, x0ps)
                            nc.vector.tensor_copy(out=x0[:, 1024:2048],
                                                  in_=x0ps[:, 1024:2048])
                    half_prelu(hA, k, hb, 0)
                    half_prelu(hB, k, hb, 1)
                    h = hb
                if nxt:
                    # L1 of macro m+1 ahead of L5: it is on the ping-pong
                    # critical path, L5/Exp only feed the (slack) store.
                    h_next = layer1(m + 1, x0)
                # L5, column-split packing: gen half H holds
                #   cols 0:512  = e  (partitions 0:64 <- h cols H*1024+0:512,
                #                      64:128 <- h cols H*1024+512:1024)
                #   cols 512:1024 = b (same split)
                # so Exp-A depends only on Prelu-A of L4. The (0,0)/(0,64)
                # col-tiled MM pairs run concurrently on the PE.
                lhsT5e = lhsT[4][:, 0:64]
                lhsT5b = lhsT[4][:, 64:128]
                for H in range(2):
                    genE = psp.tile([128, 512], FP, tag="eT", bufs=1)
                    genB = psp.tile([128, 512], FP, tag="bT", bufs=1)
                    for n in range(2):
                        src = h[:, H * 1024 + n * 512:H * 1024 + (n + 1) * 512]
                        nc.tensor.matmul(genE[64 * n:64 * n + 64, :],
                                         lhsT5e, src, start=True, stop=True)
                    for n in range(2):
                        src = h[:, H * 1024 + n * 512:H * 1024 + (n + 1) * 512]
                        nc.tensor.matmul(genB[64 * n:64 * n + 64, :],
                                         lhsT5b, src, start=True, stop=True)
                    nc.scalar.activation(
                        out=eb[:, H * 512:(H + 1) * 512], in_=genE,
                        func=mybir.ActivationFunctionType.Exp,
                        bias=biases[5], scale=1.0,
                    )
                    nc.vector.tensor_scalar_add(
                        out=eb[:, 1024 + H * 512:1024 + (H + 1) * 512],
                        in0=genB, scalar1=biases[6],
                    )
                if nxt:
                    h = h_next

                eT = psp.tile([128, 1024], BF, tag="eT", bufs=1)
                bT = psp.tile([128, 1024], BF, tag="bT", bufs=1)
                for q in range(4):
                    backT_quarter(eT, bT, eb, q)
                for Q in range(4):
                    combine_quarter(eT, bT, natbfs[m], Q)
                store(m)

            flush = sbp.tile([128, 1], FP, tag="wu")
            fl = nc.vector.tensor_copy(out=flush, in_=biases[0])
            for dma in tail_dmas:
                _add_dep_helper(fl.ins, dma.ins, sync=True,
                                reason="drain tail out-DMAs before kernel end")

    nc.finalize()
    return nc


_NC_CACHE = None


def kernel(z, ws_logs, bs_logs, ws_b, bs_b):
    global _NC_CACHE, LAST_RESULTS
    z = np.asarray(z, np.float32)
    assert z.shape == (BATCH, 16)
    consts, wmat_bf = _build_consts(ws_logs, bs_logs, ws_b, bs_b)

    if _NC_CACHE is None:
        _NC_CACHE = _build_nc()
    nc = _NC_CACHE

    in_maps = []
    for s in STARTS:
        zp = np.zeros((R + PAD_ROWS, 16), np.float32)
        zp[:R] = z[s:s + R]
        in_maps.append({"z": zp, "consts": consts, "wmat": wmat_bf})
    trace = bool(os.environ.get("AFFINE_TRACE"))
    res = run_bass_kernel_spmd(nc, in_maps, core_ids=list(range(N_CORES)), trace=trace)
    LAST_RESULTS = res

    out = np.empty((BATCH, 16), np.float32)
    for c in range(N_CORES):
        out[STARTS[c]:STARTS[c] + R] = res.results[c]["out"][:R]
    return out


# revision 20
# speedup vs baseline: 1.1448x; 1.0115x over previous
"""AffineCoupling TRN2 kernel (v6.5): 31 macros of 16384 rows per core.

Same dataflow as v7 but at 16k-row macro granularity: SWDGE cast DMAs,
transpose-mode fwd/back transposes into bf16 PSUM, A/B column-half MLP
ping-pong between PE and ACT, packed full-lane L5/Exp, in-place combine.
PSUM: hA(2) + hB(2) + x0ps(2) + eT(1) + bT(1) = 8 banks.
"""
import os
import sys

sys.path.insert(0, "/opt/trn_rl_repo")
if "/root/.axon_site/_ro/trn_rl_repo" not in sys.path:
    sys.path.append("/root/.axon_site/_ro/trn_rl_repo")

import numpy as np

import concourse.bacc as bacc
import concourse.bass as bass
import concourse.tile as tile
from concourse import mybir
from concourse.bass import _add_dep_helper
from concourse.bass_utils import run_bass_kernel_spmd

FP = mybir.dt.float32
BF = mybir.dt.bfloat16

N_CORES = 8
BATCH = 4_000_000
ROWS_PER_MACRO = 16_384
MACROS = 31
R = ROWS_PER_MACRO * MACROS        # 507,904 rows per core
PAD_ROWS = ROWS_PER_MACRO

STEP = 498_688
STARTS = [c * STEP for c in range(N_CORES - 1)] + [BATCH - R]

C_BIAS = 128
C_TOTAL = 135

LAST_RESULTS = None


def _build_consts(ws_logs, bs_logs, ws_b, bs_b):
    import ml_dtypes

    ws_logs = [np.asarray(w, np.float32) for w in ws_logs]
    bs_logs = [np.asarray(b, np.float32) for b in bs_logs]
    ws_b = [np.asarray(w, np.float32) for w in ws_b]
    bs_b = [np.asarray(b, np.float32) for b in bs_b]

    consts = np.zeros((128, C_TOTAL), np.float32)
    consts[:, 0:128] = np.eye(128, dtype=np.float32)
    for k in range(4):
        cat = np.concatenate([bs_logs[k], bs_b[k]])    # [16]
        consts[:, C_BIAS + k] = np.tile(cat, 8)
    consts[:, C_BIAS + 4] = np.concatenate(
        [np.tile(bs_logs[4], 8), np.tile(bs_b[4], 8)]
    )
    consts[:, C_BIAS + 5] = np.tile(bs_logs[4], 16)
    consts[:, C_BIAS + 6] = np.tile(bs_b[4], 16)

    wmat = np.zeros((128, 5 * 128), np.float32)
    w1cat = np.vstack([ws_logs[0], ws_b[0]])           # [16, 8]
    for g in range(8):
        wmat[g * 16:g * 16 + 8, g * 16:(g + 1) * 16] = w1cat.T
    for k in (1, 2, 3):
        wk = np.zeros((16, 16), np.float32)
        wk[0:8, 0:8] = ws_logs[k]
        wk[8:16, 8:16] = ws_b[k]
        for g in range(8):
            wmat[g * 16:(g + 1) * 16, k * 128 + g * 16:k * 128 + (g + 1) * 16] = wk.T
    for g in range(8):
        wmat[g * 16:g * 16 + 8, 4 * 128 + g * 8:4 * 128 + (g + 1) * 8] = ws_logs[4].T
        wmat[g * 16 + 8:(g + 1) * 16,
             4 * 128 + 64 + g * 8:4 * 128 + 64 + (g + 1) * 8] = ws_b[4].T
    wmat = np.concatenate([wmat, np.eye(128, dtype=np.float32)], axis=1)
    wmat_bf = wmat.astype(ml_dtypes.bfloat16)
    return consts, wmat_bf


def _ap(t, offset, dims):
    return bass.AP(tensor=t.tensor, offset=t.offset + offset, ap=[t.ap[0]] + dims)


def _build_nc():
    nc = bacc.Bacc()
    z_d = nc.declare_dram_parameter("z", [R + PAD_ROWS, 16], FP, isOutput=False)
    c_d = nc.declare_dram_parameter("consts", [128, C_TOTAL], FP, isOutput=False)
    w_d = nc.declare_dram_parameter("wmat", [128, 6 * 128], BF, isOutput=False)
    o_d = nc.declare_dram_parameter("out", [R + PAD_ROWS, 16], FP, isOutput=True)

    with tile.TileContext(nc) as tc:
        with (
            tc.tile_pool(name="consts", bufs=1) as cp,
            tc.tile_pool(name="nat", bufs=1) as natp,
            tc.tile_pool(name="sb", bufs=1) as sbp,
            tc.tile_pool(name="ps", bufs=1, space="PSUM") as psp,
        ):
            consts = cp.tile([128, C_TOTAL], FP)
            nc.sync.dma_start(out=consts, in_=c_d[:, :])
            wmat = cp.tile([128, 6 * 128], BF)
            nc.sync.dma_start(out=wmat, in_=w_d[:, :])
            identbf = wmat[:, 5 * 128:6 * 128]
            lhsT = [wmat[:, k * 128:(k + 1) * 128] for k in range(5)]
            biases = [consts[:, C_BIAS + k:C_BIAS + k + 1] for k in range(7)]

            wu1 = sbp.tile([128, 1], FP, tag="wu")
            nc.scalar.copy(out=wu1, in_=biases[0])
            wu2 = sbp.tile([128, 1], FP, tag="wu")
            nc.vector.tensor_copy(out=wu2, in_=biases[0])

            natbfs = {}
            tail_dmas = []

            def load(m):
                if m >= MACROS:
                    return
                r0 = m * ROWS_PER_MACRO
                natbf = natp.tile([128, 2048], BF, tag="nat", bufs=5)
                nc.gpsimd.dma_start(
                    out=natbf.rearrange("p (c f) -> p c f", c=128, f=16),
                    in_=z_d[r0:r0 + ROWS_PER_MACRO, :].rearrange(
                        "(p c) f -> p c f", p=128, c=128
                    ),
                )
                natbfs[m] = natbf

            def fwdT_quarter(m, q, x0ps):
                for u in range(q * 4, q * 4 + 4):
                    nc.tensor.transpose(
                        x0ps[:, u * 128:(u + 1) * 128],
                        natbfs[m][:, u * 128:(u + 1) * 128],
                        identbf,
                    )

            def half_mms(lhsT_k, h_in, half, tag):
                hps = psp.tile([128, 1024], FP, tag=tag, bufs=1)
                for n in range(2):
                    src = h_in[:, half * 1024 + n * 512:half * 1024 + (n + 1) * 512]
                    nc.tensor.matmul(hps[:, n * 512:(n + 1) * 512],
                                     lhsT_k, src, start=True, stop=True)
                return hps

            def half_prelu(hps, k, hb, half):
                nc.scalar.activation(
                    out=hb[:, half * 1024:(half + 1) * 1024], in_=hps,
                    func=mybir.ActivationFunctionType.Prelu,
                    bias=biases[k], scale=1.0, alpha=0.01,
                )

            def backT_quarter(eT, bT, eb, q):
                for u in range(q * 2, q * 2 + 2):
                    nc.tensor.transpose(
                        eT[:, u * 128:(u + 1) * 128],
                        eb[:, u * 128:(u + 1) * 128],
                        identbf,
                    )
                    nc.tensor.transpose(
                        bT[:, u * 128:(u + 1) * 128],
                        eb[:, 1024 + u * 128:1024 + (u + 1) * 128],
                        identbf,
                    )

            def combine_quarter(eT, bT, natbf, Q):
                # natbf cols [Q*512, (Q+1)*512): eT/bT offset by sub-half
                eoff = (Q // 2) * 512 + (Q % 2) * 64
                e_ap = _ap(eT, eoff, [[128, 4], [8, 8], [1, 8]])
                b_ap = _ap(bT, eoff, [[128, 4], [8, 8], [1, 8]])
                zr_ap = _ap(natbf, Q * 512 + 8, [[128, 4], [16, 8], [1, 8]])
                tmp = sbp.tile([128, 512], BF, tag="tmp", bufs=4)
                tmp_ap = _ap(tmp, 0, [[128, 4], [8, 8], [1, 8]])
                nc.vector.tensor_mul(out=tmp_ap, in0=e_ap, in1=zr_ap)
                nc.vector.tensor_add(out=zr_ap, in0=tmp_ap, in1=b_ap)

            def store(m):
                r0 = m * ROWS_PER_MACRO
                out_dma = nc.gpsimd.dma_start(
                    out=o_d[r0:r0 + ROWS_PER_MACRO, :].rearrange(
                        "(p c) f -> p c f", p=128, c=128
                    ),
                    in_=natbfs[m].rearrange("p (c f) -> p c f", c=128, f=16),
                )
                del natbfs[m]
                load(m + 3)
                if m >= MACROS - 4:
                    tail_dmas.append(out_dma)

            def layer1(m, x0):
                hb = sbp.tile([128, 2048], BF, tag="h0", bufs=3)
                hA = half_mms(lhsT[0], x0, 0, "hA")
                hB = half_mms(lhsT[0], x0, 1, "hB")
                half_prelu(hA, 0, hb, 0)
                half_prelu(hB, 0, hb, 1)
                return hb

            load(0)
            load(1)
            load(2)
            x0ps = psp.tile([128, 2048], BF, tag="x0ps", bufs=1)
            for q in range(4):
                fwdT_quarter(0, q, x0ps)
            x0 = sbp.tile([128, 2048], BF, tag="x0", bufs=3)
            nc.vector.tensor_copy(out=x0, in_=x0ps)
            h = layer1(0, x0)

            for m in range(MACROS):
                nxt = m + 1 < MACROS
                eb = sbp.tile([128, 2048], BF, tag="eb", bufs=3)
                if nxt:
                    x0ps = psp.tile([128, 2048], BF, tag="x0ps", bufs=1)
                if nxt:
                    x0 = sbp.tile([128, 2048], BF, tag="x0", bufs=3)
                for k in (1, 2, 3):
                    hb = sbp.tile([128, 2048], BF, tag=f"h{k}", bufs=3)
                    hA = half_mms(lhsT[k], h, 0, "hA")
                    hB = half_mms(lhsT[k], h, 1, "hB")
                    if nxt:
                        fwdT_quarter(m + 1, k - 1, x0ps)
                        if k == 2:
                            nc.vector.tensor_copy(out=x0[:, 0:1024],
                                                  in_=x0ps[:, 0:1024])

                    half_prelu(hA, k, hb, 0)
                    half_prelu(hB, k, hb, 1)
                    h = hb
                if nxt:
                    fwdT_quarter(m + 1, 3, x0ps)
                    nc.vector.tensor_copy(out=x0[:, 1024:2048],
                                          in_=x0ps[:, 1024:2048])
                    # L1 of macro m+1 ahead of L5: it is on the ping-pong
                    # critical path, L5/Exp only feed the (slack) store.
                    h_next = layer1(m + 1, x0)
                # L5, column-split packing: gen half H holds
                #   cols 0:512  = e  (partitions 0:64 <- h cols H*1024+0:512,
                #                      64:128 <- h cols H*1024+512:1024)
                #   cols 512:1024 = b (same split)
                # so Exp-A depends only on Prelu-A of L4. The (0,0)/(0,64)
                # col-tiled MM pairs run concurrently on the PE.
                lhsT5e = lhsT[4][:, 0:64]
                lhsT5b = lhsT[4][:, 64:128]
                for H in range(2):
                    genE = psp.tile([128, 512], FP, tag="eT", bufs=1)
                    genB = psp.tile([128, 512], FP, tag="bT", bufs=1)
                    for n in range(2):
                        src = h[:, H * 1024 + n * 512:H * 1024 + (n + 1) * 512]
                        nc.tensor.matmul(genE[64 * n:64 * n + 64, :],
                                         lhsT5e, src, start=True, stop=True)
                    for n in range(2):
                        src = h[:, H * 1024 + n * 512:H * 1024 + (n + 1) * 512]
                        nc.tensor.matmul(genB[64 * n:64 * n + 64, :],
                                         lhsT5b, src, start=True, stop=True)
                    nc.scalar.activation(
                        out=eb[:, H * 512:(H + 1) * 512], in_=genE,
                        func=mybir.ActivationFunctionType.Exp,
                        bias=biases[5], scale=1.0,
                    )
                    nc.vector.tensor_scalar_add(
                        out=eb[:, 1024 + H * 512:1024 + (H + 1) * 512],
                        in0=genB, scalar1=biases[6],
                    )
                if nxt:
                    h = h_next

                eT = psp.tile([128, 1024], BF, tag="eT", bufs=1)
                bT = psp.tile([128, 1024], BF, tag="bT", bufs=1)
                for q in range(4):
                    backT_quarter(eT, bT, eb, q)
                for Q in range(4):
                    combine_quarter(eT, bT, natbfs[m], Q)
                store(m)

            flush = sbp.tile([128, 1], FP, tag="wu")
            fl = nc.vector.tensor_copy(out=flush, in_=biases[0])
            for dma in tail_dmas:
                _add_dep_helper(fl.ins, dma.ins, sync=True,
                                reason="drain tail out-DMAs before kernel end")

    nc.finalize()
    return nc


_NC_CACHE = None


def kernel(z, ws_logs, bs_logs, ws_b, bs_b):
    global _NC_CACHE, LAST_RESULTS
    z = np.asarray(z, np.float32)
    assert z.shape == (BATCH, 16)
    consts, wmat_bf = _build_consts(ws_logs, bs_logs, ws_b, bs_b)

    if _NC_CACHE is None:
        _NC_CACHE = _build_nc()
    nc = _NC_CACHE

    in_maps = []
    for s in STARTS:
        zp = np.zeros((R + PAD_ROWS, 16), np.float32)
        zp[:R] = z[s:s + R]
        in_maps.append({"z": zp, "consts": consts, "wmat": wmat_bf})
    trace = bool(os.environ.get("AFFINE_TRACE"))
    res = run_bass_kernel_spmd(nc, in_maps, core_ids=list(range(N_CORES)), trace=trace)
    LAST_RESULTS = res

    out = np.empty((BATCH, 16), np.float32)
    for c in range(N_CORES):
        out[STARTS[c]:STARTS[c] + R] = res.results[c]["out"][:R]
    return out
